# revision 5
# baseline (speedup 1.0000x reference)
"""AttentionBlock (GroupNorm -> qkv -> 8-head attention -> proj -> residual)
on 8 Trainium2 NeuronCores, data-parallel over batch (one batch element per
core, zero collectives).

Per-core layout (B=8, C=512, H=W=32 -> S=1024, heads=8, hd=64, groups=32):
  - x, out:   (128 part, 4 c-tiles, 1024)  channel c = t*128 + p
  - GroupNorm stats via bn_stats per channel + PE matmul against a 0/1
    group-aggregation matrix for the cross-partition (16-channel) reduction;
    rstd = exp(-0.5*ln(var+eps)) so only the Exp/Ln ACT table set is used.
  - qkv: q,k computed o-major (head-dim on partitions); v computed s-major
    (V^T directly) with 64 appended all-ones columns so the P@V matmul also
    yields the softmax row-sums replicated across 64 psum partitions.
  - attention per head: S^T tile (j,i) = k^T q via one N=1024 bf16 matmul per
    j-tile; exp on ACT (no max subtraction -- logits are N(0,1), |max|<8);
    P@V accumulated over j-tiles; normalize by reciprocal(row-sums).
  - proj + residual: x (+ proj_b and the folded v-bias term W_p @ b_v) added
    during the PSUM copyback.
Matmuls run in bf16 with fp32 PSUM accumulation; GroupNorm statistics stay
fp32 end to end.
"""

import numpy as np
import ml_dtypes

import concourse.bacc as bacc
import concourse.mybir as mybir
import concourse.tile as tile
from concourse.bass_utils import run_bass_kernel_spmd

B, C, HH, WW = 8, 512, 32, 32
S = HH * WW          # 1024
HEADS, HD = 8, 64
GROUPS = 32
GSIZE = C // GROUPS  # 16 channels per group
EPS = 1e-5
P = 128
CT = C // P          # 4 channel tiles
ST = S // P          # 8 spatial tiles
QK_MT = 8            # q+k output tiles (o = 0..1023)
F32 = mybir.dt.float32
BF16 = mybir.dt.bfloat16

_NC_CACHE = {}


def build_nc(attn_reps: int = 1):
    """Build + compile the per-core Bass module.

    attn_reps > 1 repeats the whole compute body (for slope-based timing in
    test.py); the final repetition's output is the one written out.
    """
    key = attn_reps
    if key in _NC_CACHE:
        return _NC_CACHE[key]

    nc = bacc.Bacc("TRN2", target_bir_lowering=False)

    x_d = nc.dram_tensor("x", [C, S], F32, kind="ExternalInput")
    wqk_d = nc.dram_tensor("wqkT", [P, CT, 1024], BF16, kind="ExternalInput")
    wv_d = nc.dram_tensor("wvT", [P, CT, C], BF16, kind="ExternalInput")
    wp_d = nc.dram_tensor("wpT", [P, CT, C], BF16, kind="ExternalInput")
    bqk_d = nc.dram_tensor("bqk", [P, QK_MT], F32, kind="ExternalInput")
    gamma_d = nc.dram_tensor("gamma", [P, CT], F32, kind="ExternalInput")
    beta_d = nc.dram_tensor("beta", [P, CT], F32, kind="ExternalInput")
    pb_d = nc.dram_tensor("pb", [P, CT], F32, kind="ExternalInput")
    g_d = nc.dram_tensor("G", [P, GROUPS // CT], F32, kind="ExternalInput")
    gt_d = nc.dram_tensor("GT", [GROUPS // CT, P], F32, kind="ExternalInput")
    out_d = nc.dram_tensor("out", [C, S], F32, kind="ExternalOutput")

    NG = GROUPS // CT  # 8 groups per channel tile

    with tile.TileContext(nc) as tc:
        with (
            tc.tile_pool(name="const", bufs=1) as const,
            tc.tile_pool(name="work", bufs=1) as work,
            tc.tile_pool(name="small", bufs=4) as small,
            tc.tile_pool(name="expp", bufs=3) as expp,
            tc.tile_pool(name="psum", bufs=2, space="PSUM") as psum,
        ):
            # ---- constant / input loads ----
            x_sb = work.tile([P, CT, S], F32)
            nc.sync.dma_start(x_sb[:], x_d.rearrange("(t p) s -> p t s", p=P))
            wqk = const.tile([P, CT, 1024], BF16)
            nc.sync.dma_start(wqk[:], wqk_d[:])
            wv = const.tile([P, CT, C], BF16)
            nc.sync.dma_start(wv[:], wv_d[:])
            wp = const.tile([P, CT, C], BF16)
            nc.sync.dma_start(wp[:], wp_d[:])
            bqk = const.tile([P, QK_MT], F32)
            nc.sync.dma_start(bqk[:], bqk_d[:])
            gam = const.tile([P, CT], F32)
            nc.sync.dma_start(gam[:], gamma_d[:])
            bet = const.tile([P, CT], F32)
            nc.sync.dma_start(bet[:], beta_d[:])
            pb = const.tile([P, CT], F32)
            nc.sync.dma_start(pb[:], pb_d[:])
            gmat = const.tile([P, NG], F32)
            nc.sync.dma_start(gmat[:], g_d[:])
            gtmat = const.tile([NG, P], F32)
            nc.sync.dma_start(gtmat[:], gt_d[:])
            eps8 = const.tile([NG, 1], F32)
            nc.vector.memset(eps8[:], EPS)

            for rep in range(attn_reps):
                last = rep == attn_reps - 1

                # ---- GroupNorm statistics ----
                stats = small.tile([P, CT, 2], F32, tag="stats")
                for t in range(CT):
                    bst = small.tile([P, 2, 6], F32, tag="bst")
                    for half in range(2):
                        nc.vector.bn_stats(
                            bst[:, half, :],
                            x_sb[:, t, half * 512:(half + 1) * 512],
                        )
                    mv = small.tile([P, 2], F32, tag="mv")
                    nc.vector.bn_aggr(mv[:], bst[:])
                    # stats[:,t,0] = mean_c ; stats[:,t,1] = E[x^2]_c
                    nc.vector.tensor_copy(stats[:, t, 0:1], mv[:, 0:1])
                    sq = small.tile([P, 1], F32, tag="sq")
                    nc.vector.tensor_mul(sq[:], mv[:, 0:1], mv[:, 0:1])
                    nc.vector.tensor_add(stats[:, t, 1:2], mv[:, 1:2], sq[:])

                # cross-partition group sums: (NG, CT*2) = G.T @ stats
                ps_g = psum.tile([P, S], F32, tag="big")
                nc.tensor.matmul(
                    ps_g[0:NG, 0:CT * 2], gmat[:], stats[:], start=True, stop=True
                )
                gv = ps_g[0:NG, 0:CT * 2].rearrange("g (t k) -> g t k", k=2)
                bca = small.tile([NG, CT, 2], F32, tag="bca")  # [mean_g, rstd_g]
                msq = small.tile([NG, CT], F32, tag="msq")
                m2t = small.tile([NG, CT], F32, tag="m2t")
                inv = 1.0 / GSIZE  # stats are already per-channel means
                nc.vector.tensor_scalar_mul(bca[:, :, 0], gv[:, :, 0], inv)
                nc.vector.tensor_scalar_mul(msq[:], gv[:, :, 1], inv)
                nc.vector.tensor_mul(m2t[:], bca[:, :, 0], bca[:, :, 0])
                nc.vector.tensor_sub(msq[:], msq[:], m2t[:])  # var_g
                # rstd = exp(-0.5 * ln(var + eps))  (avoids the Rsqrt table set)
                nc.scalar.activation(
                    msq[:], msq[:], mybir.ActivationFunctionType.Ln, bias=eps8[:]
                )
                nc.scalar.activation(
                    bca[:, :, 1], msq[:], mybir.ActivationFunctionType.Exp,
                    scale=-0.5,
                )

                # broadcast group stats back to channels: (P, CT*2) = GT.T @ bca
                ps_c = psum.tile([P, S], F32, tag="big")
                nc.tensor.matmul(
                    ps_c[:, 0:CT * 2], gtmat[:], bca[:], start=True, stop=True
                )
                cv = ps_c[:, 0:CT * 2].rearrange("p (t k) -> p t k", k=2)
                scale_c = small.tile([P, CT], F32, tag="scale_c")
                shift_c = small.tile([P, CT], F32, tag="shift_c")
                nc.vector.tensor_mul(scale_c[:], gam[:], cv[:, :, 1])
                nc.vector.tensor_mul(shift_c[:], cv[:, :, 0], scale_c[:])
                nc.vector.tensor_sub(shift_c[:], bet[:], shift_c[:])

                # xn = x*scale + shift (bf16)
                xn = work.tile([P, CT, S], BF16, tag="xn")
                for t in range(CT):
                    nc.vector.tensor_scalar(
                        xn[:, t, :], x_sb[:, t, :],
                        scalar1=scale_c[:, t:t + 1], scalar2=shift_c[:, t:t + 1],
                        op0=mybir.AluOpType.mult, op1=mybir.AluOpType.add,
                    )

                # ---- qkv: q,k o-major ----
                qk_sb = work.tile([P, QK_MT, S], BF16, tag="qk_sb")
                for m in range(QK_MT):
                    ps = psum.tile([P, S], F32, tag="big")
                    for k in range(CT):
                        for i in range(0, S, 512):
                            nc.tensor.matmul(
                                ps[:, i:i + 512],
                                wqk[:, k, m * 128:(m + 1) * 128],
                                xn[:, k, i:i + 512],
                                start=(k == 0), stop=(k == CT - 1),
                            )
                    nc.vector.tensor_scalar_add(
                        qk_sb[:, m, :], ps[:], bqk[:, m:m + 1]
                    )

                # ---- v: s-major (V^T) + all-ones columns for row-sums ----
                vT = work.tile([P, ST, HEADS, 2 * HD], BF16, tag="vT")
                nc.vector.memset(vT[:, :, :, HD:2 * HD], 1.0)
                for s in range(ST):
                    ps = psum.tile([P, S], F32, tag="big")
                    for k in range(CT):
                        nc.tensor.matmul(
                            ps[:, 0:C], xn[:, k, s * 128:(s + 1) * 128],
                            wv[:, k, :],
                            start=(k == 0), stop=(k == CT - 1),
                        )
                    nc.vector.tensor_copy(
                        vT[:, s, :, 0:HD],
                        ps[:, 0:C].rearrange("p (h d) -> p h d", d=HD),
                    )

                # x += proj_b_eff (for the residual; after xn was computed)
                for t in range(CT):
                    nc.vector.tensor_scalar_add(
                        x_sb[:, t, :], x_sb[:, t, :], pb[:, t:t + 1]
                    )

                # ---- attention, one head at a time ----
                a_sb = work.tile([P, CT, S], BF16, tag="a_sb")
                for h in range(HEADS):
                    po = (h % 2) * HD
                    mq = h // 2
                    mk = 4 + h // 2
                    ps_pv = psum.tile([P, S], F32, tag="pv")
                    eps_t = []
                    for jt in range(ST):
                        ps_st = psum.tile([P, S], F32, tag="big")
                        for i in range(0, S, 512):
                            nc.tensor.matmul(
                                ps_st[:, i:i + 512],
                                qk_sb[po:po + HD, mk, jt * 128:(jt + 1) * 128],
                                qk_sb[po:po + HD, mq, i:i + 512],
                                start=True, stop=True,
                            )
                        ep = expp.tile([P, S], BF16, tag="expp")
                        nc.scalar.activation(
                            ep[:], ps_st[:], mybir.ActivationFunctionType.Exp
                        )
                        for i in range(0, S, 512):
                            nc.tensor.matmul(
                                ps_pv[:, i:i + 512], vT[:, jt, h, :],
                                ep[:, i:i + 512],
                                start=(jt == 0), stop=(jt == ST - 1),
                                skip_group_check=True,
                            )
                    rec = small.tile([HD, S], F32, tag="rec")
                    nc.vector.reciprocal(rec[:], ps_pv[HD:2 * HD, :])
                    nc.vector.tensor_mul(
                        a_sb[po:po + HD, h // 2, :], ps_pv[0:HD, :], rec[:]
                    )

                # ---- proj + residual ----
                out_sb = work.tile([P, CT, S], F32, tag="out_sb")
                for m in range(CT):
                    ps = psum.tile([P, S], F32, tag="pv")
                    for k in range(CT):
                        for i in range(0, S, 512):
                            nc.tensor.matmul(
                                ps[:, i:i + 512],
                                wp[:, k, m * 128:(m + 1) * 128],
                                a_sb[:, k, i:i + 512],
                                start=(k == 0), stop=(k == CT - 1),
                            )
                    nc.vector.tensor_add(out_sb[:, m, :], ps[:], x_sb[:, m, :])

                if last:
                    nc.sync.dma_start(
                        out_d.rearrange("(t p) s -> p t s", p=P), out_sb[:]
                    )
                else:
                    # un-do the x += pb so the next repetition is identical
                    for t in range(CT):
                        nc.vector.tensor_scalar(
                            x_sb[:, t, :], x_sb[:, t, :],
                            scalar1=pb[:, t:t + 1], scalar2=None,
                            op0=mybir.AluOpType.subtract,
                        )

    nc.compile()
    _NC_CACHE[key] = nc
    return nc


def _prep_weights(norm_w, norm_b, qkv_w, qkv_b, proj_w, proj_b):
    f32 = np.float32
    bf16 = ml_dtypes.bfloat16
    qkv_w = np.asarray(qkv_w, f32)
    qkv_b = np.asarray(qkv_b, f32)
    proj_w = np.asarray(proj_w, f32)
    proj_b = np.asarray(proj_b, f32)
    sc = 1.0 / np.sqrt(HD).astype(f32)

    wqk = np.concatenate([qkv_w[:C] * sc, qkv_w[C:2 * C]], axis=0)  # (1024, C)
    wqkT = np.ascontiguousarray(
        wqk.T.reshape(CT, P, 1024).transpose(1, 0, 2)
    ).astype(bf16)
    wvT = np.ascontiguousarray(
        qkv_w[2 * C:].T.reshape(CT, P, C).transpose(1, 0, 2)
    ).astype(bf16)
    wpT = np.ascontiguousarray(
        proj_w.T.reshape(CT, P, C).transpose(1, 0, 2)
    ).astype(bf16)
    bqk = np.ascontiguousarray(
        np.concatenate([qkv_b[:C] * sc, qkv_b[C:2 * C]]).reshape(QK_MT, P).T
    ).astype(f32)
    pb_eff = proj_b + proj_w @ qkv_b[2 * C:]
    pb = np.ascontiguousarray(pb_eff.reshape(CT, P).T).astype(f32)
    gamma = np.ascontiguousarray(np.asarray(norm_w, f32).reshape(CT, P).T)
    beta = np.ascontiguousarray(np.asarray(norm_b, f32).reshape(CT, P).T)
    G = (np.arange(P)[:, None] // GSIZE == np.arange(GROUPS // CT)[None, :])
    G = np.ascontiguousarray(G.astype(f32))
    GT = np.ascontiguousarray(G.T)
    return dict(wqkT=wqkT, wvT=wvT, wpT=wpT, bqk=bqk, pb=pb,
                gamma=gamma, beta=beta, G=G, GT=GT)


def kernel(x, norm_w, norm_b, qkv_w, qkv_b, proj_w, proj_b, _attn_reps=1,
           _results_only=True):
    x = np.asarray(x, np.float32)
    shared = _prep_weights(norm_w, norm_b, qkv_w, qkv_b, proj_w, proj_b)
    in_maps = [
        {"x": np.ascontiguousarray(x[b].reshape(C, S)), **shared}
        for b in range(B)
    ]
    nc = build_nc(_attn_reps)
    res = run_bass_kernel_spmd(nc, in_maps, core_ids=list(range(B)))
    out = np.stack([res.results[b]["out"] for b in range(B)])
    return out.reshape(B, C, HH, WW).astype(np.float32)
